# revision 26
# baseline (speedup 1.0000x reference)
"""DynamicMultiLinear (MoE-style grouped linear) Trainium2 kernel.

Problem: y[i] = x[i] @ W[g(i)].T + b[g(i)] where rows of x are contiguous
segments per network g (G=256 networks, IN=OUT=256, N=262144 rows).

Strategy (expert-parallel, per sharding hint):
  - Shard the group axis over 8 cores: core d owns networks [32d, 32d+32)
    and their contiguous row segments of x.
  - Host packs, per core, a transposed + 128-padded activation tensor
    xp[j, p, k, r] = x_seg[r, k*128+p] so the device only does dense,
    statically-shaped work (same program on all 8 cores; pad count B =
    ceil(max(counts)/128) blocks per network is a compile-time constant).
  - Device: for each network j, yT[m-chunk] = sum_k W^T[k,m-chunk].T @ xT[k]
    via fp32 matmuls accumulated in PSUM, bias fused into the PSUM->SBUF
    copy on the scalar engine.
  - Host unpads/transposes back to y[N, 256].
"""

import sys
from contextlib import ExitStack

import numpy as np

if "/opt/trn_rl_repo" not in sys.path:
    sys.path.insert(0, "/opt/trn_rl_repo")

G = 256
IN_F = 256
OUT_F = 256
N_CORES = 8
GPC = G // N_CORES  # networks per core
# "fp32":  exact fp32 matmuls (~7e-7 absmax-rel, slowest)
# "hilo3": bf16 hi/lo split of x and W, 3 products (~5e-6, fast)
# "bf16":  single bf16 product (~2.3e-3, fastest)
MODE = "hilo3"


def _row_groups(B):
    """Split B 128-row blocks into matmul row-group sizes <= 512 (fp32 moving
    operand limit), as few groups as possible."""
    groups = []
    rem = B
    while rem > 0:
        take = min(rem, 4)
        groups.append(take * 128)
        rem -= take
    return groups


def _split_multi_waits(bir):
    """The walrus build here supports only ONE sem-wait per instruction
    (setupSyncWait: 'Too many sync wait commands'). Hoist extra waits onto
    same-engine NoOps inserted directly before the instruction — engines
    dispatch their stream in order, so gating semantics are preserved."""
    changed = False
    for fn in bir["functions"]:
        for bb in fn["blocks"]:
            out = []
            for ins in bb["instructions"]:
                si = ins.get("sync_info")
                waits = (si or {}).get("on_wait") or []
                if len(waits) > 1:
                    changed = True
                    for i, w in enumerate(waits[:-1]):
                        out.append(
                            {
                                "debug": ins.get("debug", 0),
                                "engine": ins["engine"],
                                "ins": [],
                                "name": f"{ins['name']}-sw{i}",
                                "opcode": "NoOp",
                                "outs": [],
                                "sync_info": {"on_update": [], "on_wait": [w]},
                                "text_hint": "split_wait",
                            }
                        )
                    si["on_wait"] = [waits[-1]]
                out.append(ins)
            bb["instructions"] = out
    return changed


def _patch_to_json():
    import json

    import concourse.bass as bass

    if getattr(bass.Bass, "_split_waits_patched", False):
        return
    orig = bass.Bass.to_json_bytes

    def to_json_bytes(self):
        data = orig(self)
        bir = json.loads(data)
        if _split_multi_waits(bir):
            data = json.dumps(bir).encode()
        return data

    bass.Bass.to_json_bytes = to_json_bytes
    bass.Bass._split_waits_patched = True


def _patch_tile_drain():
    """The walrus build in this container rejects sem waits on InstDrain
    ("Too many sync wait commands", CoreV3GenImpl setupSyncWait). Re-emit the
    TileContext exit drain's waits as NOPs (which do accept waits) ahead of a
    wait-free drain — same sequencer, same semantics."""
    import bass_rust
    from concourse import tile
    from concourse.tile import ScopedClock

    if getattr(tile.TileContext, "_drain_patched", False):
        return

    def _drain_and_barrier(self, tick_clock, wait_clock):
        gc = tick_clock.global_clock
        procs = [i for i in range(27) if gc[i] > 0]
        maxw = 1
        for i0 in range(0, len(procs), maxw):
            nop = self.nc.sync.nop(nofuse=True, hint="predrain_wait")
            masked = bass_rust.VectorClock()
            for i in procs[i0 : i0 + maxw]:
                masked.require_at_least(i, gc[i])
            wait_clock.add_sem_waits(nop.ins, ScopedClock({None: masked}))
        self.nc.sync.drain()
        self.nc.all_engine_barrier()
        popped = self.nc._tile_sem_poison_stack.pop()
        assert popped is self._sem_poison
        self.nc.clear_and_free_semaphores(list(self.sems.allocated().values()))
        self.nc.all_engine_barrier()

    tile.TileContext._drain_and_barrier = _drain_and_barrier
    tile.TileContext._drain_patched = True


def _build_program(B, gpc=GPC, mode=None):
    import concourse.bass as bass
    import concourse.mybir as mybir
    from concourse import tile

    _patch_tile_drain()
    _patch_to_json()

    mode = mode or MODE
    R = B * 128
    f32 = mybir.dt.float32
    mdt = f32 if mode == "fp32" else mybir.dt.bfloat16
    # (x_plane, w_plane) products accumulated per psum tile, per k-chunk.
    # hilo3: x = xh + xl, W = Wh + Wl (bf16 planes); xh*Wh + xl*Wh + xh*Wl.
    prods = [(0, 0), (1, 0), (0, 1)] if mode == "hilo3" else [(0, 0)]
    PX = 1 + max(p[0] for p in prods)  # x planes
    PW = 1 + max(p[1] for p in prods)  # w planes
    XF = 2 * PX * R  # x tile free size: [k][xplane][R]
    WF = 2 * PW * 256  # per-net w free size: [k][wplane][o]

    nc = bass.Bass(target_bir_lowering=False)
    xp = nc.declare_dram_parameter("xp", [gpc, 128, XF], mdt, isOutput=False)
    wt = nc.declare_dram_parameter("wt", [128, gpc * WF], mdt, isOutput=False)
    bt = nc.declare_dram_parameter("bt", [128, gpc * 2], f32, isOutput=False)
    yp = nc.declare_dram_parameter("yp", [gpc, 128, 2 * R], f32, isOutput=True)

    rgs = _row_groups(B)
    # per-k matmul sequence, grouped so consecutive matmuls share the
    # stationary operand (one weight reload per group, pipelined by PE)
    by_w = {}
    for xpl, wpl in prods:
        by_w.setdefault(wpl, []).append(xpl)
    combos = []  # (k, x_plane, w_plane) in issue order
    for k in range(2):
        for wpl, xpls in by_w.items():
            for xpl in xpls:
                combos.append((k, xpl, wpl))

    with ExitStack() as ctx:
        tc = ctx.enter_context(tile.TileContext(nc))
        wpool = ctx.enter_context(tc.tile_pool(name="w", bufs=1))
        bpool = ctx.enter_context(tc.tile_pool(name="b", bufs=1))
        # keep pools inside SBUF for any B (weights take gpc*WF*dt fixed)
        x_bytes = XF * (2 if mdt != f32 else 4)  # per partition
        y_bytes = 2 * R * 4
        x_bufs = max(2, min(7, 57344 // x_bytes))
        y_bufs = max(2, min(4, 38912 // y_bytes))
        xpool = ctx.enter_context(tc.tile_pool(name="x", bufs=x_bufs))
        ypool = ctx.enter_context(tc.tile_pool(name="y", bufs=y_bufs))
        pspool = ctx.enter_context(tc.tile_pool(name="ps", bufs=8, space="PSUM"))

        # weight quads (~1MB DMAs) interleaved into the network loop so the
        # first matmuls are gated only on the first quad + first x tile
        quad = 4
        while gpc % quad:
            quad //= 2
        w_quads = [None] * (gpc // quad)

        def load_quad(q):
            w_q = wpool.tile([128, quad * WF], mdt, tag=f"w{q}", name=f"w{q}")
            nc.sync.dma_start(w_q[:], wt[:, q * quad * WF : (q + 1) * quad * WF])
            w_quads[q] = w_q

        b_all = bpool.tile([128, gpc * 2], f32)

        yp4 = yp.rearrange("j p (m r) -> j p m r", m=2)
        for j in range(gpc):
            x_t = xpool.tile([128, XF], mdt)
            if j == 0:
                # split the first load so the first matmul (k=0 chunk) is
                # gated on half the bytes
                nc.sync.dma_start(x_t[:, : XF // 2], xp[j][:, : XF // 2])
                nc.sync.dma_start(x_t[:, XF // 2 :], xp[j][:, XF // 2 :])
            else:
                nc.sync.dma_start(x_t[:], xp[j])
            if j % quad == 0:
                load_quad(j // quad)
            if j == 0:
                nc.sync.dma_start(b_all[:], bt[:])
            w_j = w_quads[j // quad][:, (j % quad) * WF : (j % quad + 1) * WF]
            y_t = ypool.tile([128, 2 * R], f32)
            for m in range(2):
                pss = [
                    pspool.tile([128, rg], f32, tag="ps", name="ps") for rg in rgs
                ]
                for ci, (k, xpl, wpl) in enumerate(combos):
                    w0 = k * PW * 256 + wpl * 256 + m * 128
                    x0 = k * PX * R + xpl * R
                    r0 = 0
                    for ps, rg in zip(pss, rgs):
                        nc.tensor.matmul(
                            ps[:],
                            w_j[:, w0 : w0 + 128],
                            x_t[:, x0 + r0 : x0 + r0 + rg],
                            start=(ci == 0),
                            stop=(ci == len(combos) - 1),
                        )
                        r0 += rg
                r0 = 0
                for ps, rg in zip(pss, rgs):
                    bias_ap = b_all[:, 2 * j + m : 2 * j + m + 1]
                    y_slice = y_t[:, m * R + r0 : m * R + r0 + rg]
                    # split the PSUM->SBUF bias-copy between ScalarE and DVE
                    # so neither becomes the bottleneck
                    if m == 0:
                        nc.scalar.activation(
                            y_slice,
                            ps[:],
                            mybir.ActivationFunctionType.Identity,
                            bias=bias_ap,
                        )
                    else:
                        nc.vector.tensor_scalar_add(y_slice, ps[:], bias_ap)
                    r0 += rg
                # per-m output DMA so the kernel tail is one half-tile deep;
                # issued via SWDGE (idle Pool engine) so store waits don't
                # head-of-line block the SP sequencer's x/w prefetch DMAs
                nc.gpsimd.dma_start(yp4[j, :, m, :], y_t[:, m * R : (m + 1) * R])
    return nc


_PROGRAM_CACHE = {}


def _get_program(B):
    key = (B, MODE)
    if key not in _PROGRAM_CACHE:
        _PROGRAM_CACHE[key] = _build_program(B)
    return _PROGRAM_CACHE[key]


def _planes(a, n_planes):
    """Split fp32 array into bf16 hi/lo planes (or keep fp32 for 1 plane)."""
    if MODE == "fp32":
        return [a]
    import ml_dtypes

    bf = np.dtype(ml_dtypes.bfloat16)
    hi = a.astype(bf)
    if n_planes == 1:
        return [hi]
    lo = (a - hi.astype(np.float32)).astype(bf)
    return [hi, lo]


def _pack_inputs(weight, bias, x, counts, offs, B):
    """Build the 8 per-core input maps (transpose + pad on host)."""
    if MODE == "fp32":
        mdt = np.dtype(np.float32)
    else:
        import ml_dtypes

        mdt = np.dtype(ml_dtypes.bfloat16)
    PX = 2 if MODE == "hilo3" else 1
    PW = 2 if MODE == "hilo3" else 1
    R = B * 128
    in_maps = []
    for d in range(N_CORES):
        xp = np.zeros((GPC, 128, 2, PX, R), mdt)
        for j in range(GPC):
            g = d * GPC + j
            c = int(counts[g])
            if c:
                seg = x[offs[g] : offs[g] + c]  # [c, 256]
                for pl, sp in enumerate(_planes(seg, PX)):
                    # [p, k, r] <- seg[r, k*128+p]
                    xp[j, :, :, pl, :c] = (
                        sp.T.reshape(2, 128, c).transpose(1, 0, 2).astype(mdt)
                    )
        W = weight[d * GPC : (d + 1) * GPC]  # [GPC, OUT, IN]
        wt = np.empty((128, GPC, 2, PW, OUT_F), mdt)
        for pl, wp in enumerate(_planes(W, PW)):
            # [i, j, k, o] <- W[j, o, k*128+i]
            wt[:, :, :, pl, :] = (
                wp.reshape(GPC, OUT_F, 2, 128).transpose(3, 0, 2, 1).astype(mdt)
            )
        bt = bias[d * GPC : (d + 1) * GPC, 0, :].reshape(GPC, 2, 128)
        bt = bt.transpose(2, 0, 1)  # [128, GPC, 2]
        in_maps.append(
            {
                "xp": np.ascontiguousarray(xp.reshape(GPC, 128, 2 * PX * R)),
                "wt": np.ascontiguousarray(wt.reshape(128, GPC * 2 * PW * OUT_F)),
                "bt": np.ascontiguousarray(bt.reshape(128, GPC * 2)),
            }
        )
    return in_maps


def _unpack_outputs(results, counts, offs, B, n):
    R = B * 128
    y = np.empty((n, OUT_F), np.float32)
    for d in range(N_CORES):
        ypd = np.asarray(results[d]["yp"]).reshape(GPC, 128, 2, R)
        for j in range(GPC):
            g = d * GPC + j
            c = int(counts[g])
            if c:
                y[offs[g] : offs[g] + c] = (
                    ypd[j, :, :, :c].transpose(1, 0, 2).reshape(256, c).T
                )
    return y


def kernel(weight, bias, x, counts):
    from concourse.bass_utils import run_bass_kernel_spmd

    weight = np.ascontiguousarray(np.asarray(weight), dtype=np.float32)
    bias = np.ascontiguousarray(np.asarray(bias), dtype=np.float32)
    x = np.ascontiguousarray(np.asarray(x), dtype=np.float32)
    counts = np.asarray(counts).astype(np.int64)
    n = x.shape[0]
    offs = np.zeros(G + 1, np.int64)
    np.cumsum(counts, out=offs[1:])
    B = max(1, -(-int(counts.max()) // 128))

    nc = _get_program(B)
    in_maps = _pack_inputs(weight, bias, x, counts, offs, B)
    res = run_bass_kernel_spmd(nc, in_maps, list(range(N_CORES)))
    return _unpack_outputs(res.results, counts, offs, B, n)


# revision 28
# speedup vs baseline: 1.0858x; 1.0858x over previous
"""DynamicMultiLinear (MoE-style grouped linear) Trainium2 kernel.

Problem: y[i] = x[i] @ W[g(i)].T + b[g(i)] where rows of x are contiguous
segments per network g (G=256 networks, IN=OUT=256, N=262144 rows).

Strategy (expert-parallel, per sharding hint):
  - Shard the group axis over 8 cores: core d owns networks [32d, 32d+32)
    and their contiguous row segments of x.
  - Host packs, per core, a transposed + 128-padded activation tensor
    xp[j, p, k, r] = x_seg[r, k*128+p] so the device only does dense,
    statically-shaped work (same program on all 8 cores; pad count B =
    ceil(max(counts)/128) blocks per network is a compile-time constant).
  - Device: for each network j, yT[m-chunk] = sum_k W^T[k,m-chunk].T @ xT[k]
    accumulated in PSUM. Default mode "hilo3" splits x and W into bf16
    hi/lo planes and accumulates 3 bf16 products (xh*Wh + xl*Wh + xh*Wl)
    -> fp32-class accuracy (~5e-6 absmax-rel) at bf16 matmul throughput.
    Bias is fused into the PSUM->SBUF copy (ScalarE/VectorE alternating);
    outputs stream back over the GpSimd SWDGE queue so stores never block
    the SP sequencer's prefetch DMAs.
  - Host unpads/transposes back to y[N, 256].

Measured on trn2 (8 cores): ~256 us HW exec, absmax-rel err 4.8e-6
(vs fp32 reference). Pure-fp32 mode measures ~282 us @ 7e-7; pure-bf16
~180 us @ 2.3e-3.
"""

import sys
from contextlib import ExitStack

import numpy as np

if "/opt/trn_rl_repo" not in sys.path:
    sys.path.insert(0, "/opt/trn_rl_repo")

G = 256
IN_F = 256
OUT_F = 256
N_CORES = 8
GPC = G // N_CORES  # networks per core
# "fp32":  exact fp32 matmuls (~7e-7 absmax-rel, slowest)
# "hilo3": bf16 hi/lo split of x and W, 3 products (~5e-6, fast)
# "bf16":  single bf16 product (~2.3e-3, fastest)
MODE = "hilo3"


def _row_groups(B):
    """Split B 128-row blocks into matmul row-group sizes <= 512 (fp32 moving
    operand limit), as few groups as possible."""
    groups = []
    rem = B
    while rem > 0:
        take = min(rem, 4)
        groups.append(take * 128)
        rem -= take
    return groups


def _split_multi_waits(bir):
    """The walrus build here supports only ONE sem-wait per instruction
    (setupSyncWait: 'Too many sync wait commands'). Hoist extra waits onto
    same-engine NoOps inserted directly before the instruction — engines
    dispatch their stream in order, so gating semantics are preserved."""
    changed = False
    for fn in bir["functions"]:
        for bb in fn["blocks"]:
            out = []
            for ins in bb["instructions"]:
                si = ins.get("sync_info")
                waits = (si or {}).get("on_wait") or []
                if len(waits) > 1:
                    changed = True
                    for i, w in enumerate(waits[:-1]):
                        out.append(
                            {
                                "debug": ins.get("debug", 0),
                                "engine": ins["engine"],
                                "ins": [],
                                "name": f"{ins['name']}-sw{i}",
                                "opcode": "NoOp",
                                "outs": [],
                                "sync_info": {"on_update": [], "on_wait": [w]},
                                "text_hint": "split_wait",
                            }
                        )
                    si["on_wait"] = [waits[-1]]
                out.append(ins)
            bb["instructions"] = out
    return changed


def _patch_to_json():
    import json

    import concourse.bass as bass

    if getattr(bass.Bass, "_split_waits_patched", False):
        return
    orig = bass.Bass.to_json_bytes

    def to_json_bytes(self):
        data = orig(self)
        bir = json.loads(data)
        if _split_multi_waits(bir):
            data = json.dumps(bir).encode()
        return data

    bass.Bass.to_json_bytes = to_json_bytes
    bass.Bass._split_waits_patched = True


def _patch_tile_drain():
    """The walrus build in this container rejects sem waits on InstDrain
    ("Too many sync wait commands", CoreV3GenImpl setupSyncWait). Re-emit the
    TileContext exit drain's waits as NOPs (which do accept waits) ahead of a
    wait-free drain — same sequencer, same semantics."""
    import bass_rust
    from concourse import tile
    from concourse.tile import ScopedClock

    if getattr(tile.TileContext, "_drain_patched", False):
        return

    def _drain_and_barrier(self, tick_clock, wait_clock):
        gc = tick_clock.global_clock
        procs = [i for i in range(27) if gc[i] > 0]
        maxw = 1
        for i0 in range(0, len(procs), maxw):
            nop = self.nc.sync.nop(nofuse=True, hint="predrain_wait")
            masked = bass_rust.VectorClock()
            for i in procs[i0 : i0 + maxw]:
                masked.require_at_least(i, gc[i])
            wait_clock.add_sem_waits(nop.ins, ScopedClock({None: masked}))
        self.nc.sync.drain()
        self.nc.all_engine_barrier()
        popped = self.nc._tile_sem_poison_stack.pop()
        assert popped is self._sem_poison
        self.nc.clear_and_free_semaphores(list(self.sems.allocated().values()))
        self.nc.all_engine_barrier()

    tile.TileContext._drain_and_barrier = _drain_and_barrier
    tile.TileContext._drain_patched = True


def _build_program(B, gpc=GPC, mode=None):
    import concourse.bass as bass
    import concourse.mybir as mybir
    from concourse import tile

    _patch_tile_drain()
    _patch_to_json()

    mode = mode or MODE
    R = B * 128
    f32 = mybir.dt.float32
    mdt = f32 if mode == "fp32" else mybir.dt.bfloat16
    # (x_plane, w_plane) products accumulated per psum tile, per k-chunk.
    # hilo3: x = xh + xl, W = Wh + Wl (bf16 planes); xh*Wh + xl*Wh + xh*Wl.
    prods = [(0, 0), (1, 0), (0, 1)] if mode == "hilo3" else [(0, 0)]
    PX = 1 + max(p[0] for p in prods)  # x planes
    PW = 1 + max(p[1] for p in prods)  # w planes
    XF = 2 * PX * R  # x tile free size: [k][xplane][R]
    WF = 2 * PW * 256  # per-net w free size: [k][wplane][o]

    nc = bass.Bass(target_bir_lowering=False)
    xp = nc.declare_dram_parameter("xp", [gpc, 128, XF], mdt, isOutput=False)
    wt = nc.declare_dram_parameter("wt", [128, gpc * WF], mdt, isOutput=False)
    bt = nc.declare_dram_parameter("bt", [128, gpc * 2], f32, isOutput=False)
    yp = nc.declare_dram_parameter("yp", [gpc, 128, 2 * R], f32, isOutput=True)

    rgs = _row_groups(B)
    # per-k matmul sequence, grouped so consecutive matmuls share the
    # stationary operand (one weight reload per group, pipelined by PE)
    by_w = {}
    for xpl, wpl in prods:
        by_w.setdefault(wpl, []).append(xpl)
    combos = []  # (k, x_plane, w_plane) in issue order
    for k in range(2):
        for wpl, xpls in by_w.items():
            for xpl in xpls:
                combos.append((k, xpl, wpl))

    with ExitStack() as ctx:
        tc = ctx.enter_context(tile.TileContext(nc))
        wpool = ctx.enter_context(tc.tile_pool(name="w", bufs=1))
        bpool = ctx.enter_context(tc.tile_pool(name="b", bufs=1))
        # keep pools inside SBUF for any B (weights take gpc*WF*dt fixed)
        x_bytes = XF * (2 if mdt != f32 else 4)  # per partition
        y_bytes = 2 * R * 4
        x_bufs = max(2, min(7, 65536 // x_bytes))
        y_bufs = max(2, min(4, 38912 // y_bytes))
        xpool = ctx.enter_context(tc.tile_pool(name="x", bufs=x_bufs))
        ypool = ctx.enter_context(tc.tile_pool(name="y", bufs=y_bufs))
        pspool = ctx.enter_context(tc.tile_pool(name="ps", bufs=8, space="PSUM"))

        # weight quads (~1MB DMAs) interleaved into the network loop so the
        # first matmuls are gated only on the first quad + first x tile
        quad = 4
        while gpc % quad:
            quad //= 2
        w_quads = [None] * (gpc // quad)

        def load_quad(q):
            w_q = wpool.tile([128, quad * WF], mdt, tag=f"w{q}", name=f"w{q}")
            nc.sync.dma_start(w_q[:], wt[:, q * quad * WF : (q + 1) * quad * WF])
            w_quads[q] = w_q

        b_all = bpool.tile([128, gpc * 2], f32)

        yp4 = yp.rearrange("j p (m r) -> j p m r", m=2)
        for j in range(gpc):
            x_t = xpool.tile([128, XF], mdt)
            if j == 0:
                # split the first load so the first matmul (k=0 chunk) is
                # gated on half the bytes
                nc.sync.dma_start(x_t[:, : XF // 2], xp[j][:, : XF // 2])
                nc.sync.dma_start(x_t[:, XF // 2 :], xp[j][:, XF // 2 :])
            else:
                nc.sync.dma_start(x_t[:], xp[j])
            if j % quad == 0:
                load_quad(j // quad)
            if j == 0:
                nc.sync.dma_start(b_all[:], bt[:])
            w_j = w_quads[j // quad][:, (j % quad) * WF : (j % quad + 1) * WF]
            y_t = ypool.tile([128, 2 * R], f32)
            for m in range(2):
                pss = [
                    pspool.tile([128, rg], f32, tag="ps", name="ps") for rg in rgs
                ]
                for ci, (k, xpl, wpl) in enumerate(combos):
                    w0 = k * PW * 256 + wpl * 256 + m * 128
                    x0 = k * PX * R + xpl * R
                    r0 = 0
                    for ps, rg in zip(pss, rgs):
                        nc.tensor.matmul(
                            ps[:],
                            w_j[:, w0 : w0 + 128],
                            x_t[:, x0 + r0 : x0 + r0 + rg],
                            start=(ci == 0),
                            stop=(ci == len(combos) - 1),
                        )
                        r0 += rg
                r0 = 0
                for ps, rg in zip(pss, rgs):
                    bias_ap = b_all[:, 2 * j + m : 2 * j + m + 1]
                    y_slice = y_t[:, m * R + r0 : m * R + r0 + rg]
                    # split the PSUM->SBUF bias-copy between ScalarE and DVE
                    # so neither becomes the bottleneck
                    if m == 0:
                        nc.scalar.activation(
                            y_slice,
                            ps[:],
                            mybir.ActivationFunctionType.Identity,
                            bias=bias_ap,
                        )
                    else:
                        nc.vector.tensor_scalar_add(y_slice, ps[:], bias_ap)
                    r0 += rg
                # per-m output DMA so the kernel tail is one half-tile deep;
                # issued via SWDGE (idle Pool engine) so store waits don't
                # head-of-line block the SP sequencer's x/w prefetch DMAs
                nc.gpsimd.dma_start(yp4[j, :, m, :], y_t[:, m * R : (m + 1) * R])
    return nc


_PROGRAM_CACHE = {}


def _get_program(B):
    key = (B, MODE)
    if key not in _PROGRAM_CACHE:
        _PROGRAM_CACHE[key] = _build_program(B)
    return _PROGRAM_CACHE[key]


def _planes(a, n_planes):
    """Split fp32 array into bf16 hi/lo planes (or keep fp32 for 1 plane)."""
    if MODE == "fp32":
        return [a]
    import ml_dtypes

    bf = np.dtype(ml_dtypes.bfloat16)
    hi = a.astype(bf)
    if n_planes == 1:
        return [hi]
    lo = (a - hi.astype(np.float32)).astype(bf)
    return [hi, lo]


def _pack_inputs(weight, bias, x, counts, offs, B):
    """Build the 8 per-core input maps (transpose + pad on host)."""
    if MODE == "fp32":
        mdt = np.dtype(np.float32)
    else:
        import ml_dtypes

        mdt = np.dtype(ml_dtypes.bfloat16)
    PX = 2 if MODE == "hilo3" else 1
    PW = 2 if MODE == "hilo3" else 1
    R = B * 128
    in_maps = []
    for d in range(N_CORES):
        xp = np.zeros((GPC, 128, 2, PX, R), mdt)
        for j in range(GPC):
            g = d * GPC + j
            c = int(counts[g])
            if c:
                seg = x[offs[g] : offs[g] + c]  # [c, 256]
                for pl, sp in enumerate(_planes(seg, PX)):
                    # [p, k, r] <- seg[r, k*128+p]
                    xp[j, :, :, pl, :c] = (
                        sp.T.reshape(2, 128, c).transpose(1, 0, 2).astype(mdt)
                    )
        W = weight[d * GPC : (d + 1) * GPC]  # [GPC, OUT, IN]
        wt = np.empty((128, GPC, 2, PW, OUT_F), mdt)
        for pl, wp in enumerate(_planes(W, PW)):
            # [i, j, k, o] <- W[j, o, k*128+i]
            wt[:, :, :, pl, :] = (
                wp.reshape(GPC, OUT_F, 2, 128).transpose(3, 0, 2, 1).astype(mdt)
            )
        bt = bias[d * GPC : (d + 1) * GPC, 0, :].reshape(GPC, 2, 128)
        bt = bt.transpose(2, 0, 1)  # [128, GPC, 2]
        in_maps.append(
            {
                "xp": np.ascontiguousarray(xp.reshape(GPC, 128, 2 * PX * R)),
                "wt": np.ascontiguousarray(wt.reshape(128, GPC * 2 * PW * OUT_F)),
                "bt": np.ascontiguousarray(bt.reshape(128, GPC * 2)),
            }
        )
    return in_maps


def _unpack_outputs(results, counts, offs, B, n):
    R = B * 128
    y = np.empty((n, OUT_F), np.float32)
    for d in range(N_CORES):
        ypd = np.asarray(results[d]["yp"]).reshape(GPC, 128, 2, R)
        for j in range(GPC):
            g = d * GPC + j
            c = int(counts[g])
            if c:
                y[offs[g] : offs[g] + c] = (
                    ypd[j, :, :, :c].transpose(1, 0, 2).reshape(256, c).T
                )
    return y


def kernel(weight, bias, x, counts):
    from concourse.bass_utils import run_bass_kernel_spmd

    weight = np.ascontiguousarray(np.asarray(weight), dtype=np.float32)
    bias = np.ascontiguousarray(np.asarray(bias), dtype=np.float32)
    x = np.ascontiguousarray(np.asarray(x), dtype=np.float32)
    counts = np.asarray(counts).astype(np.int64)
    n = x.shape[0]
    offs = np.zeros(G + 1, np.int64)
    np.cumsum(counts, out=offs[1:])
    B = max(1, -(-int(counts.max()) // 128))

    nc = _get_program(B)
    in_maps = _pack_inputs(weight, bias, x, counts, offs, B)
    res = run_bass_kernel_spmd(nc, in_maps, list(range(N_CORES)))
    return _unpack_outputs(res.results, counts, offs, B, n)


# revision 30
# speedup vs baseline: 1.0863x; 1.0004x over previous
"""DynamicMultiLinear (MoE-style grouped linear) Trainium2 kernel.

Problem: y[i] = x[i] @ W[g(i)].T + b[g(i)] where rows of x are contiguous
segments per network g (G=256 networks, IN=OUT=256, N=262144 rows).

Strategy (expert-parallel, per sharding hint):
  - Shard the group axis over 8 cores: core d owns networks [32d, 32d+32)
    and their contiguous row segments of x.
  - Host packs, per core, a transposed + 128-padded activation tensor
    xp[j, p, k, r] = x_seg[r, k*128+p] so the device only does dense,
    statically-shaped work (same program on all 8 cores; pad count B =
    ceil(max(counts)/128) blocks per network is a compile-time constant).
  - Device: for each network j, yT[m-chunk] = sum_k W^T[k,m-chunk].T @ xT[k]
    accumulated in PSUM. Default mode "hilo3" splits x and W into bf16
    hi/lo planes and accumulates 3 bf16 products (xh*Wh + xl*Wh + xh*Wl)
    -> fp32-class accuracy (~5e-6 absmax-rel) at bf16 matmul throughput.
    Bias is fused into the PSUM->SBUF copy (ScalarE/VectorE alternating);
    outputs stream back over the GpSimd SWDGE queue so stores never block
    the SP sequencer's prefetch DMAs.
  - Host unpads/transposes back to y[N, 256].

Measured on trn2 (8 cores): ~256 us HW exec, absmax-rel err 4.8e-6
(vs fp32 reference). Pure-fp32 mode measures ~282 us @ 7e-7; pure-bf16
~180 us @ 2.3e-3.
"""

import sys
from contextlib import ExitStack

import numpy as np

if "/opt/trn_rl_repo" not in sys.path:
    sys.path.insert(0, "/opt/trn_rl_repo")

G = 256
IN_F = 256
OUT_F = 256
N_CORES = 8
GPC = G // N_CORES  # networks per core
# "fp32":  exact fp32 matmuls (~7e-7 absmax-rel, slowest)
# "hilo3": bf16 hi/lo split of x and W, 3 products (~5e-6, fast)
# "bf16":  single bf16 product (~2.3e-3, fastest)
MODE = "hilo3"


def _row_groups(B):
    """Split B 128-row blocks into matmul row-group sizes <= 512 (fp32 moving
    operand limit), as few groups as possible."""
    groups = []
    rem = B
    while rem > 0:
        take = min(rem, 4)
        groups.append(take * 128)
        rem -= take
    return groups


def _split_multi_waits(bir):
    """The walrus build here supports only ONE sem-wait per instruction
    (setupSyncWait: 'Too many sync wait commands'). Hoist extra waits onto
    same-engine NoOps inserted directly before the instruction — engines
    dispatch their stream in order, so gating semantics are preserved."""
    changed = False
    for fn in bir["functions"]:
        for bb in fn["blocks"]:
            out = []
            for ins in bb["instructions"]:
                si = ins.get("sync_info")
                waits = (si or {}).get("on_wait") or []
                if len(waits) > 1:
                    changed = True
                    for i, w in enumerate(waits[:-1]):
                        out.append(
                            {
                                "debug": ins.get("debug", 0),
                                "engine": ins["engine"],
                                "ins": [],
                                "name": f"{ins['name']}-sw{i}",
                                "opcode": "NoOp",
                                "outs": [],
                                "sync_info": {"on_update": [], "on_wait": [w]},
                                "text_hint": "split_wait",
                            }
                        )
                    si["on_wait"] = [waits[-1]]
                out.append(ins)
            bb["instructions"] = out
    return changed


def _patch_to_json():
    import json

    import concourse.bass as bass

    if getattr(bass.Bass, "_split_waits_patched", False):
        return
    orig = bass.Bass.to_json_bytes

    def to_json_bytes(self):
        data = orig(self)
        bir = json.loads(data)
        if _split_multi_waits(bir):
            data = json.dumps(bir).encode()
        return data

    bass.Bass.to_json_bytes = to_json_bytes
    bass.Bass._split_waits_patched = True


def _patch_tile_drain():
    """The walrus build in this container rejects sem waits on InstDrain
    ("Too many sync wait commands", CoreV3GenImpl setupSyncWait). Re-emit the
    TileContext exit drain's waits as NOPs (which do accept waits) ahead of a
    wait-free drain — same sequencer, same semantics."""
    import bass_rust
    from concourse import tile
    from concourse.tile import ScopedClock

    if getattr(tile.TileContext, "_drain_patched", False):
        return

    def _drain_and_barrier(self, tick_clock, wait_clock):
        gc = tick_clock.global_clock
        procs = [i for i in range(27) if gc[i] > 0]
        maxw = 1
        for i0 in range(0, len(procs), maxw):
            nop = self.nc.sync.nop(nofuse=True, hint="predrain_wait")
            masked = bass_rust.VectorClock()
            for i in procs[i0 : i0 + maxw]:
                masked.require_at_least(i, gc[i])
            wait_clock.add_sem_waits(nop.ins, ScopedClock({None: masked}))
        self.nc.sync.drain()
        self.nc.all_engine_barrier()
        popped = self.nc._tile_sem_poison_stack.pop()
        assert popped is self._sem_poison
        self.nc.clear_and_free_semaphores(list(self.sems.allocated().values()))
        self.nc.all_engine_barrier()

    tile.TileContext._drain_and_barrier = _drain_and_barrier
    tile.TileContext._drain_patched = True


def _build_program(B, gpc=GPC, mode=None):
    import concourse.bass as bass
    import concourse.mybir as mybir
    from concourse import tile

    _patch_tile_drain()
    _patch_to_json()

    mode = mode or MODE
    R = B * 128
    f32 = mybir.dt.float32
    mdt = f32 if mode == "fp32" else mybir.dt.bfloat16
    # (x_plane, w_plane) products accumulated per psum tile, per k-chunk.
    # hilo3: x = xh + xl, W = Wh + Wl (bf16 planes); xh*Wh + xl*Wh + xh*Wl.
    prods = [(0, 0), (1, 0), (0, 1)] if mode == "hilo3" else [(0, 0)]
    PX = 1 + max(p[0] for p in prods)  # x planes
    PW = 1 + max(p[1] for p in prods)  # w planes
    XF = 2 * PX * R  # x tile free size: [k][xplane][R]
    WF = 2 * PW * 256  # per-net w free size: [k][wplane][o]

    nc = bass.Bass(target_bir_lowering=False)
    xp = nc.declare_dram_parameter("xp", [gpc, 128, XF], mdt, isOutput=False)
    wt = nc.declare_dram_parameter("wt", [128, gpc * WF], mdt, isOutput=False)
    bt = nc.declare_dram_parameter("bt", [128, gpc * 2], f32, isOutput=False)
    yp = nc.declare_dram_parameter("yp", [gpc, 128, 2 * R], f32, isOutput=True)

    rgs = _row_groups(B)
    # per-k matmul sequence, grouped so consecutive matmuls share the
    # stationary operand (one weight reload per group, pipelined by PE)
    by_w = {}
    for xpl, wpl in prods:
        by_w.setdefault(wpl, []).append(xpl)
    combos = []  # (k, x_plane, w_plane) in issue order
    for k in range(2):
        for wpl, xpls in by_w.items():
            for xpl in xpls:
                combos.append((k, xpl, wpl))

    with ExitStack() as ctx:
        tc = ctx.enter_context(tile.TileContext(nc))
        wpool = ctx.enter_context(tc.tile_pool(name="w", bufs=1))
        bpool = ctx.enter_context(tc.tile_pool(name="b", bufs=1))
        # keep pools inside SBUF for any B (weights take gpc*WF*dt fixed)
        x_bytes = XF * (2 if mdt != f32 else 4)  # per partition
        y_bytes = 2 * R * 4
        x_bufs = max(2, min(7, 65536 // x_bytes))
        y_bufs = max(2, min(4, 38912 // y_bytes))
        xpool = ctx.enter_context(tc.tile_pool(name="x", bufs=x_bufs))
        ypool = ctx.enter_context(tc.tile_pool(name="y", bufs=y_bufs))
        pspool = ctx.enter_context(tc.tile_pool(name="ps", bufs=8, space="PSUM"))

        # weight quads (~1MB DMAs) interleaved into the network loop so the
        # first matmuls are gated only on the first quad + first x tile
        quad = 4
        while gpc % quad:
            quad //= 2
        w_quads = [None] * (gpc // quad)

        def load_quad(q):
            w_q = wpool.tile([128, quad * WF], mdt, tag=f"w{q}", name=f"w{q}")
            if q == 0 and quad > 1:
                # net 0's weights first so the very first matmul isn't gated
                # on the whole quad
                nc.sync.dma_start(w_q[:, :WF], wt[:, :WF])
                nc.sync.dma_start(w_q[:, WF:], wt[:, WF : quad * WF])
            else:
                nc.sync.dma_start(w_q[:], wt[:, q * quad * WF : (q + 1) * quad * WF])
            w_quads[q] = w_q

        b_all = bpool.tile([128, gpc * 2], f32)

        yp4 = yp.rearrange("j p (m r) -> j p m r", m=2)
        for j in range(gpc):
            x_t = xpool.tile([128, XF], mdt)
            if j in (0, gpc - 1):
                # split first load (shorter pipeline fill) and last load
                # (k=0 matmuls start before the k=1 half lands -> shorter tail)
                nc.sync.dma_start(x_t[:, : XF // 2], xp[j][:, : XF // 2])
                nc.sync.dma_start(x_t[:, XF // 2 :], xp[j][:, XF // 2 :])
            else:
                nc.sync.dma_start(x_t[:], xp[j])
            if j % quad == 0:
                load_quad(j // quad)
            if j == 0:
                nc.sync.dma_start(b_all[:], bt[:])
            w_j = w_quads[j // quad][:, (j % quad) * WF : (j % quad + 1) * WF]
            y_t = ypool.tile([128, 2 * R], f32)
            for m in range(2):
                pss = [
                    pspool.tile([128, rg], f32, tag="ps", name="ps") for rg in rgs
                ]
                for ci, (k, xpl, wpl) in enumerate(combos):
                    w0 = k * PW * 256 + wpl * 256 + m * 128
                    x0 = k * PX * R + xpl * R
                    r0 = 0
                    for ps, rg in zip(pss, rgs):
                        nc.tensor.matmul(
                            ps[:],
                            w_j[:, w0 : w0 + 128],
                            x_t[:, x0 + r0 : x0 + r0 + rg],
                            start=(ci == 0),
                            stop=(ci == len(combos) - 1),
                        )
                        r0 += rg
                r0 = 0
                for ps, rg in zip(pss, rgs):
                    bias_ap = b_all[:, 2 * j + m : 2 * j + m + 1]
                    y_slice = y_t[:, m * R + r0 : m * R + r0 + rg]
                    # split the PSUM->SBUF bias-copy between ScalarE and DVE
                    # so neither becomes the bottleneck
                    if m == 0:
                        nc.scalar.activation(
                            y_slice,
                            ps[:],
                            mybir.ActivationFunctionType.Identity,
                            bias=bias_ap,
                        )
                    else:
                        nc.vector.tensor_scalar_add(y_slice, ps[:], bias_ap)
                    r0 += rg
                # per-m output DMA so the kernel tail is one half-tile deep;
                # issued via SWDGE (idle Pool engine) so store waits don't
                # head-of-line block the SP sequencer's x/w prefetch DMAs
                nc.gpsimd.dma_start(yp4[j, :, m, :], y_t[:, m * R : (m + 1) * R])
    return nc


_PROGRAM_CACHE = {}


def _get_program(B):
    key = (B, MODE)
    if key not in _PROGRAM_CACHE:
        _PROGRAM_CACHE[key] = _build_program(B)
    return _PROGRAM_CACHE[key]


def _planes(a, n_planes):
    """Split fp32 array into bf16 hi/lo planes (or keep fp32 for 1 plane)."""
    if MODE == "fp32":
        return [a]
    import ml_dtypes

    bf = np.dtype(ml_dtypes.bfloat16)
    hi = a.astype(bf)
    if n_planes == 1:
        return [hi]
    lo = (a - hi.astype(np.float32)).astype(bf)
    return [hi, lo]


def _pack_inputs(weight, bias, x, counts, offs, B):
    """Build the 8 per-core input maps (transpose + pad on host)."""
    if MODE == "fp32":
        mdt = np.dtype(np.float32)
    else:
        import ml_dtypes

        mdt = np.dtype(ml_dtypes.bfloat16)
    PX = 2 if MODE == "hilo3" else 1
    PW = 2 if MODE == "hilo3" else 1
    R = B * 128
    in_maps = []
    for d in range(N_CORES):
        xp = np.zeros((GPC, 128, 2, PX, R), mdt)
        for j in range(GPC):
            g = d * GPC + j
            c = int(counts[g])
            if c:
                seg = x[offs[g] : offs[g] + c]  # [c, 256]
                for pl, sp in enumerate(_planes(seg, PX)):
                    # [p, k, r] <- seg[r, k*128+p]
                    xp[j, :, :, pl, :c] = (
                        sp.T.reshape(2, 128, c).transpose(1, 0, 2).astype(mdt)
                    )
        W = weight[d * GPC : (d + 1) * GPC]  # [GPC, OUT, IN]
        wt = np.empty((128, GPC, 2, PW, OUT_F), mdt)
        for pl, wp in enumerate(_planes(W, PW)):
            # [i, j, k, o] <- W[j, o, k*128+i]
            wt[:, :, :, pl, :] = (
                wp.reshape(GPC, OUT_F, 2, 128).transpose(3, 0, 2, 1).astype(mdt)
            )
        bt = bias[d * GPC : (d + 1) * GPC, 0, :].reshape(GPC, 2, 128)
        bt = bt.transpose(2, 0, 1)  # [128, GPC, 2]
        in_maps.append(
            {
                "xp": np.ascontiguousarray(xp.reshape(GPC, 128, 2 * PX * R)),
                "wt": np.ascontiguousarray(wt.reshape(128, GPC * 2 * PW * OUT_F)),
                "bt": np.ascontiguousarray(bt.reshape(128, GPC * 2)),
            }
        )
    return in_maps


def _unpack_outputs(results, counts, offs, B, n):
    R = B * 128
    y = np.empty((n, OUT_F), np.float32)
    for d in range(N_CORES):
        ypd = np.asarray(results[d]["yp"]).reshape(GPC, 128, 2, R)
        for j in range(GPC):
            g = d * GPC + j
            c = int(counts[g])
            if c:
                y[offs[g] : offs[g] + c] = (
                    ypd[j, :, :, :c].transpose(1, 0, 2).reshape(256, c).T
                )
    return y


def kernel(weight, bias, x, counts):
    from concourse.bass_utils import run_bass_kernel_spmd

    weight = np.ascontiguousarray(np.asarray(weight), dtype=np.float32)
    bias = np.ascontiguousarray(np.asarray(bias), dtype=np.float32)
    x = np.ascontiguousarray(np.asarray(x), dtype=np.float32)
    counts = np.asarray(counts).astype(np.int64)
    n = x.shape[0]
    offs = np.zeros(G + 1, np.int64)
    np.cumsum(counts, out=offs[1:])
    B = max(1, -(-int(counts.max()) // 128))

    nc = _get_program(B)
    in_maps = _pack_inputs(weight, bias, x, counts, offs, B)
    res = run_bass_kernel_spmd(nc, in_maps, list(range(N_CORES)))
    return _unpack_outputs(res.results, counts, offs, B, n)
